# revision 1
# baseline (speedup 1.0000x reference)
"""Trainium2 Bass kernel for the combined point-cloud loss (chamfer + EMD-surrogate + conf).

Strategy (8 NeuronCores, data parallel):
  core = 2*b + h  handles batch b, half h of the up/radar points (full gt set).
  Distance tiles are produced on the PE as a single fp16 split-precision K=13
  matmul (hi/lo fp16 splitting of coords and squared norms keeps ~5e-5 abs
  accuracy on squared distances while running the PE at 1 cycle/row).
  ScalarE (ACT) applies relu + fp32->fp16 cast PSUM->SBUF.
  VectorE (DVE) does the two min passes (row mins for dist1/emd/conf and a
  running column-min accumulator for dist2) in fp16 2x mode.
  Per-core partial sums / partial column mins are combined on the host
  (the cheap "all-reduce" step of the data-parallel sharding).
"""

import numpy as np

import concourse.bacc as bacc
import concourse.bass as bass
import concourse.tile as tile
from concourse import mybir
from concourse.bass_utils import run_bass_kernel_spmd

F16 = mybir.dt.float16
F32 = mybir.dt.float32
MIN = mybir.AluOpType.min
ADD = mybir.AluOpType.add
MULT = mybir.AluOpType.mult
SUB = mybir.AluOpType.subtract
AX = mybir.AxisListType.X
AF = mybir.ActivationFunctionType

B = 4
N_UP = 8192
N_GT = 8192
N_RAD = 1024
HALF_UP = N_UP // 2      # 4096 up points per core
HALF_RAD = N_RAD // 2    # 512 radar points per core
UP_TILES = HALF_UP // 128    # 32
RAD_TILES = HALF_RAD // 128  # 4
GT_GROUPS = N_GT // 2048     # 4 psum-sized gt column groups
N_CORES = 8

_NC_CACHE = {}


def _build_nc(loop_n=1):
    from contextlib import ExitStack

    nc = bacc.Bacc("TRN2")
    up_p = nc.declare_dram_parameter("up_lhsT", [13, HALF_UP], F16, isOutput=False)
    rad_p = nc.declare_dram_parameter("rad_lhsT", [13, HALF_RAD], F16, isOutput=False)
    gt_p = nc.declare_dram_parameter("gt_rhs", [13, N_GT], F16, isOutput=False)
    conf_p = nc.declare_dram_parameter("conf_t", [128, RAD_TILES], F32, isOutput=False)
    ident_p = nc.declare_dram_parameter("ident", [128, 128], F16, isOutput=False)
    d2_p = nc.declare_dram_parameter("d2_out", [128, N_GT // 128], F32, isOutput=True)
    row_p = nc.declare_dram_parameter("row_out", [128, 3], F32, isOutput=True)

    with ExitStack() as ctx:
        tc = ctx.enter_context(tile.TileContext(nc))
        singles = ctx.enter_context(tc.tile_pool(name="singles", bufs=1))
        psum = ctx.enter_context(tc.tile_pool(name="psum", bufs=2, space="PSUM"))
        stage = ctx.enter_context(tc.tile_pool(name="stage", bufs=2))
        folds = ctx.enter_context(tc.tile_pool(name="folds", bufs=2))
        smalls = ctx.enter_context(tc.tile_pool(name="smalls", bufs=2))

        up_sb = singles.tile([13, HALF_UP], F16)
        rad_sb = singles.tile([13, HALF_RAD], F16)
        gt_sb = singles.tile([13, N_GT], F16)
        conf_sb = singles.tile([128, RAD_TILES], F32)
        ident_sb = singles.tile([128, 128], F16)
        nc.sync.dma_start(out=up_sb, in_=up_p[:])
        nc.sync.dma_start(out=rad_sb, in_=rad_p[:])
        nc.sync.dma_start(out=gt_sb, in_=gt_p[:])
        nc.sync.dma_start(out=conf_sb, in_=conf_p[:])
        nc.sync.dma_start(out=ident_sb, in_=ident_p[:])

        colacc = singles.tile([128, N_GT], F16)
        minsq = singles.tile([128, UP_TILES], F32)
        minsq_rad = singles.tile([128, RAD_TILES], F32)
        row_sums = singles.tile([128, 3], F32)

        loop_ctx = tc.For_i(0, loop_n, 1) if loop_n > 1 else None
        if loop_ctx is not None:
            ctx.enter_context(loop_ctx)

        def dist_tile(lhsT, dst_min, update_colacc, first):
            # Full [128 x N_GT] fp16 clamped distance block for one A-side tile.
            # The first up-tile's casts write straight into colacc (saves the
            # 8192-wide init copy); its rowmin folds read colacc instead.
            st = colacc if first else stage.tile([128, N_GT], F16, tag="stage")
            for jg in range(GT_GROUPS):
                ps = psum.tile([128, 2048], F32, tag="ps")
                for jj in range(4):
                    c0 = jg * 2048 + jj * 512
                    nc.tensor.matmul(
                        ps[:, jj * 512 : (jj + 1) * 512],
                        lhsT=lhsT,
                        rhs=gt_sb[:, c0 : c0 + 512],
                        start=True,
                        stop=True,
                    )
                nc.scalar.activation(
                    out=st[:, jg * 2048 : (jg + 1) * 2048], in_=ps[:], func=AF.Relu
                )
            if update_colacc and not first:
                nc.vector.tensor_tensor(colacc, colacc, st, MIN)
            # log2 folds along free dim, then a final 512-wide reduce
            f1 = folds.tile([128, 4096], F16, tag="f1")
            nc.vector.tensor_tensor(f1, st[:, :4096], st[:, 4096:], MIN)
            f2 = folds.tile([128, 2048], F16, tag="f2")
            nc.vector.tensor_tensor(f2, f1[:, :2048], f1[:, 2048:], MIN)
            f3 = folds.tile([128, 1024], F16, tag="f3")
            nc.vector.tensor_tensor(f3, f2[:, :1024], f2[:, 1024:], MIN)
            f4 = folds.tile([128, 512], F16, tag="f4")
            nc.vector.tensor_tensor(f4, f3[:, :512], f3[:, 512:], MIN)
            nc.vector.tensor_reduce(dst_min, f4, axis=AX, op=MIN)

        for i in range(UP_TILES):
            dist_tile(
                up_sb[:, i * 128 : (i + 1) * 128],
                minsq[:, i : i + 1],
                update_colacc=True,
                first=(i == 0),
            )
        for t in range(RAD_TILES):
            dist_tile(
                rad_sb[:, t * 128 : (t + 1) * 128],
                minsq_rad[:, t : t + 1],
                update_colacc=False,
                first=False,
            )

        # dist2 partition-axis min: PE-transpose 128x128 blocks of colacc into
        # PSUM (gt on partitions), then free-axis reduce_min 4 blocks at a time.
        d2t = singles.tile([128, N_GT // 128], F32)
        for tq in range(N_GT // 1024):
            tp = psum.tile([128, 1024], F16, tag="ps")
            for tt in range(8):
                blk = tq * 8 + tt
                nc.tensor.transpose(
                    tp[:, tt * 128 : (tt + 1) * 128],
                    colacc[:, blk * 128 : (blk + 1) * 128],
                    ident_sb,
                )
            nc.vector.tensor_reduce(
                d2t[:, tq * 8 : (tq + 1) * 8],
                tp.rearrange("p (b f) -> p b f", f=128),
                axis=AX,
                op=MIN,
            )
        nc.sync.dma_start(out=d2_p[:], in_=d2t)

        # dist1 sum and emd (sum of sqrt)
        nc.vector.tensor_reduce(row_sums[:, 0:1], minsq, axis=AX, op=ADD)
        sqrt_t = smalls.tile([128, UP_TILES], F32, tag="sqrt")
        nc.scalar.sqrt(sqrt_t, minsq)
        nc.vector.tensor_reduce(row_sums[:, 1:2], sqrt_t, axis=AX, op=ADD)

        # conf loss partials: score = exp(-sqrt(minsq_rad)); sse over free dim
        sr = smalls.tile([128, RAD_TILES], F32, tag="sr")
        nc.scalar.sqrt(sr, minsq_rad)
        sc = smalls.tile([128, RAD_TILES], F32, tag="sc")
        nc.scalar.activation(out=sc, in_=sr, func=AF.Exp, scale=-1.0)
        diff = smalls.tile([128, RAD_TILES], F32, tag="diff")
        nc.vector.tensor_tensor(diff, conf_sb, sc, SUB)
        dsq = smalls.tile([128, RAD_TILES], F32, tag="dsq")
        nc.vector.tensor_tensor(dsq, diff, diff, MULT)
        nc.vector.tensor_reduce(row_sums[:, 2:3], dsq, axis=AX, op=ADD)

        nc.sync.dma_start(out=row_p[:], in_=row_sums)

    nc.compile()
    return nc


def _get_nc():
    if "nc" not in _NC_CACHE:
        _NC_CACHE["nc"] = _build_nc()
    return _NC_CACHE["nc"]


def _split16(x):
    h = x.astype(np.float16)
    l = (x.astype(np.float64) - h.astype(np.float64)).astype(np.float16)
    return h, l


def _build_A(pts):
    # pts [N,3] fp32 -> lhsT [13, N] fp16
    n = pts.shape[0]
    ah, al = _split16(pts)
    a2 = np.sum(pts.astype(np.float64) ** 2, axis=1)
    a2h, a2l = _split16(a2)
    out = np.empty((13, n), dtype=np.float16)
    out[0:3] = ah.T
    out[3:6] = al.T
    out[6:9] = ah.T
    out[9] = a2h
    out[10] = a2l
    out[11] = 1.0
    out[12] = 1.0
    return out


def _build_B(pts):
    # pts [M,3] fp32 -> rhs [13, M] fp16
    m = pts.shape[0]
    bh, bl = _split16(pts)
    b2 = np.sum(pts.astype(np.float64) ** 2, axis=1)
    b2h, b2l = _split16(b2)
    out = np.empty((13, m), dtype=np.float16)
    out[0:3] = -2.0 * bh.T
    out[3:6] = -2.0 * bh.T
    out[6:9] = -2.0 * bl.T
    out[9] = 1.0
    out[10] = 1.0
    out[11] = b2h
    out[12] = b2l
    return out


def _make_in_maps(pc_up, pc_conf, pc2, pc3):
    ident = np.eye(128, dtype=np.float16)
    in_maps = []
    for core in range(N_CORES):
        b, h = divmod(core, 2)
        up = pc_up[b, h * HALF_UP : (h + 1) * HALF_UP]
        rad = pc3[b, h * HALF_RAD : (h + 1) * HALF_RAD]
        conf = pc_conf[b, h * HALF_RAD : (h + 1) * HALF_RAD, 0]
        in_maps.append(
            {
                "up_lhsT": _build_A(up),
                "rad_lhsT": _build_A(rad),
                "gt_rhs": _build_B(pc2[b]),
                "conf_t": np.ascontiguousarray(
                    conf.reshape(RAD_TILES, 128).T.astype(np.float32)
                ),
                "ident": ident,
            }
        )
    return in_maps


def kernel(pc_up, pc_seed, pc_conf, pc2, pc3):
    del pc_seed  # unused by the reference loss
    nc = _get_nc()
    in_maps = _make_in_maps(pc_up, pc_conf, pc2, pc3)
    results = run_bass_kernel_spmd(nc, in_maps, list(range(N_CORES))).results

    # Host-side combine (the "all-reduce" of the data-parallel sharding).
    tot_d1 = 0.0
    tot_sqrt = 0.0
    tot_d2 = 0.0
    tot_sse = 0.0
    for b in range(B):
        r0 = results[2 * b]
        r1 = results[2 * b + 1]
        # d2_out[p, t] corresponds to gt index t*128 + p
        d2 = np.minimum(
            r0["d2_out"].T.astype(np.float64), r1["d2_out"].T.astype(np.float64)
        )
        tot_d2 += d2.sum()
        for r in (r0, r1):
            row = r["row_out"].astype(np.float64)
            tot_d1 += row[:, 0].sum()
            tot_sqrt += row[:, 1].sum()
            tot_sse += row[:, 2].sum()

    m1 = tot_d1 / (B * N_UP)
    m2 = tot_d2 / (B * N_GT)
    emd = tot_sqrt / (B * N_UP)
    conf_mse = tot_sse / (B * N_RAD)
    alpha = 0.5
    chamfer = 0.5 * m1 + 2.0 * m2
    final = alpha * chamfer + alpha * conf_mse + emd
    return np.array(final, dtype=np.float32)

